# revision 72
# baseline (speedup 1.0000x reference)
"""GAT (graph attention network) forward pass on 8 Trainium2 NeuronCores.

Problem: nn_GAT - N=4096 nodes, F=512 features, H=8 heads, 1% dense adjacency.
    heads:  Wh = x @ Ws[h]; e = lrelu(s1[i]+s2[j]); att = masked softmax; elu(att @ Wh)
    out layer: same attention structure on hcat @ Wo, then elu.

Strategy (row-sharded across 8 cores, 3 launches):
  k0: each core computes Wh (all heads, one fused f16 matmul per 128-row
      chunk) + raw/exp'd score vectors for its 512 nodes; host gathers.
  k1: each core runs 8-head masked-softmax attention for its 512 query rows.
      Key identities: exp(lrelu(e)) = max(exp(e), exp(0.2e)); exp(e) factors
      rank-1 as exp(s1)[i]*exp(s2)[j]; and softmax tolerates any per-query-row
      scale, so scaling row i by exp(-s1[i]) turns both branches into
      per-partition scalars -- the NxN tiles need NO transcendentals:
        w = (f1b * f2[j]) max e2[j]   (one 2-op tensor_scalar, DVE 4x mode)
        p = w * adjT                  (mask multiply, fp16)
      Tiles are routed across three engine pipelines (see _k1_route), the
      jb-outer loop is software-pipelined one block deep, and six heads
      accumulate in out[i,c] orientation (65-col matmuls) with the psum
      banks zeroed up front because start=True resets bank-sharing siblings.
      The softmax denominator comes free as a ones-column in the value
      matrix; ELU uses elu(x) = min(exp(x)-1, relu(x)).
      Also computes hcat @ Wo (+ output-layer score vectors) for its rows.
  k2: output-layer attention for the core's 512 rows (same masked-softmax
      structure, one head of 513 value columns); final ELU.

adj is passed from host as a pre-transposed fp16 (exact for a 0/1 mask) slice
per core; x/weights arrive pre-transposed f16 (pure layout prep, no FLOPs).
"""

import sys

for _p in ("/opt/trn_rl_repo",):
    if _p not in sys.path:
        sys.path.insert(0, _p)

import numpy as np

import concourse.bass as bass
import concourse.tile as tile
from concourse import bacc, mybir
from concourse.bass_utils import run_bass_kernel_spmd
from concourse.masks import make_identity

N, F, H, NH = 4096, 512, 8, 64
M = 8            # cores
R = N // M       # 512 query rows per core
JB = N // 128    # 32 key blocks
IC = R // 128    # 4 query-row chunks per core
HC = NH + 1      # 65 value cols per head (64 + ones col for row sums)
ALPHA = 0.2     # leaky relu slope
BIG = 200.0     # additive mask; 0.2*BIG=40 so masked exp underflows to 0
f32 = mybir.dt.float32
f16 = mybir.dt.float16
OP = mybir.AluOpType
AF = mybir.ActivationFunctionType

_CACHE = {}


def _run(nc, in_maps, core_ids, tries=3):
    """run_bass_kernel_spmd with retry: the axon-tunneled devices
    occasionally report NRT_EXEC_UNIT_UNRECOVERABLE transiently."""
    import time as _time

    for attempt in range(tries):
        try:
            return run_bass_kernel_spmd(nc, in_maps, core_ids=core_ids)
        except Exception:
            if attempt == tries - 1:
                raise
            _time.sleep(5.0)


# ---------------------------------------------------------------- k0
def _build_k0():
    """Per-core: Wh projection for this core's R nodes, all heads fused.

    in:  xin [F, R+F+2H] f16 ([xT | Ws stacked | ws1 ws2 cols], host concat)
    out: wha16 [R, H*HC] f16 (per head: 64 value cols + ones col)
         ev [R, 5H] f32 (exp(s2) | exp(.2 s2) | exp(-.8 s1) | s1 | s2 per head)
    """
    XW = R + F + 2 * H
    nc = bacc.Bacc("TRN2", target_bir_lowering=False, debug=False, num_devices=M)
    # xin = [xT | Ws stacked | ws1 ws2 cols] concatenated on host: one DMA
    # generation per 128-row chunk covers data and weights together
    xin = nc.dram_tensor("xin", [F, XW], f16, kind="ExternalInput").ap()
    wha16 = nc.dram_tensor("wha16", [R, H * HC], f16, kind="ExternalOutput").ap()
    ev = nc.dram_tensor("ev", [R, 5 * H], f32, kind="ExternalOutput").ap()

    with tile.TileContext(nc) as tc:
        with (
            tc.tile_pool(name="sb", bufs=1) as sb,
            tc.tile_pool(name="ps", bufs=1, space="PSUM") as ps,
            tc.tile_pool(name="ob", bufs=4) as ob,
        ):
            xin_v = xin.rearrange("(c p) w -> p c w", p=128)
            xwb = []
            for fc in range(4):
                t = sb.tile([128, XW], f16, tag=f"xwb{fc}", name=f"xwb{fc}")
                nc.sync.dma_start(out=t, in_=xin_v[:, fc, :])
                xwb.append(t)
            evall = sb.tile([128, 4, 5 * H], f32, tag="evall")
            # fc-outer accumulation: all 8 psum chains live, so PE starts
            # after the first fc chunk lands and never waits again
            whps = [ps.tile([128, F], f32, tag=f"whp{nb}", name=f"whp{nb}")
                    for nb in range(IC)]
            svps = [ps.tile([128, 2 * H], f32, tag=f"svp{nb}", name=f"svp{nb}")
                    for nb in range(IC)]
            for fc in range(4):
                xt = xwb[fc][:, 0:R]
                ws = xwb[fc][:, R:R + F]
                wv = xwb[fc][:, R + F:XW]
                st, sp = (fc == 0), (fc == 3)
                for nb in range(IC):
                    nc.tensor.matmul(
                        whps[nb], xt[:, nb * 128:(nb + 1) * 128], ws,
                        start=st, stop=sp,
                    )
                    nc.tensor.matmul(
                        svps[nb], xt[:, nb * 128:(nb + 1) * 128], wv,
                        start=st, stop=sp,
                    )
                    if sp:
                        # finalize nb as soon as its chain stops
                        whp, svp = whps[nb], svps[nb]
                        wt = ob.tile([128, H * HC], f16, tag="wt")
                        wt_v = wt.rearrange("p (h c) -> p h c", c=HC)
                        nc.vector.tensor_copy(
                            wt_v[:, :, 0:NH],
                            whp.rearrange("p (h c) -> p h c", c=NH),
                        )
                        nc.vector.memset(wt_v[:, :, NH], 1.0)
                        nc.sync.dma_start(
                            out=wha16[nb * 128:(nb + 1) * 128, :], in_=wt
                        )
                        evt = evall[:, nb, :]
                        # sv cols: 0..H-1 = s1, H..2H-1 = s2 per head
                        nc.scalar.activation(evt[:, 0:H], svp[:, H:2 * H], AF.Exp)
                        nc.scalar.activation(
                            evt[:, H:2 * H], svp[:, H:2 * H], AF.Exp, scale=0.2
                        )
                        nc.scalar.activation(
                            evt[:, 2 * H:3 * H], svp[:, 0:H], AF.Exp, scale=-0.8
                        )
                        nc.vector.tensor_copy(evt[:, 3 * H:5 * H], svp[:, 0:2 * H])
            nc.sync.dma_start(
                out=ev.rearrange("(b p) c -> p b c", p=128), in_=evall
            )
    nc.compile()
    return nc


# ---------------------------------------------------------------- k1
# Tile routing: per (head, key-block) the masked-exp tile is built on one of
# three engine pipelines, ratios chosen from the TimelineSim cost model
# (DVE pair ~499ns, Pool pair ~1917ns, ACT additive-mask route ~1224ns):
#   'd' (DVE):  w = (f1b*f2) max e2 [TS 4x], p = w*adjT [TT 2x]
#   'p' (Pool): same two ops on GPSIMD
#   'a' (ACT):  e' = s1-BIG + BIG*adjT [PE->PSUM], p = exp(prelu(e'+s2)) [ACT]
# Heads 0-5 accumulate orientation-B (out[i,c]; 65-col matmuls, 4 per tile);
# heads 6-7 accumulate orientation-A (outT[c,i]; one 512-col matmul) so the
# PSUM budget is exactly 8 banks: 4x acc_ic[128,390] + 2x outT[65,512] +
# 2x eps[128,512] double-buffered.
A_HEADS = (6, 7)


def _k1_route(h, jb):
    # NOTE: the 'a' route produces p at a different per-row scale than
    # 'd'/'p' (which fold in exp(-s1[i])); softmax cancels a per-row scale
    # only if every key-block of a head agrees, so a head is either all-'a'
    # or a d/p mix -- never both.
    if h in A_HEADS:
        return 'a'
    return 'p' if (h * JB + jb) % 14 < 3 else 'd'


def _build_k1():
    """Per-core: 8-head attention for this core's R query rows + Who projection.

    in:  wha  [N, H*HC] f16 (full, from k0)
         evf  [N, 5H]  f32 (full)
         evmy [R, 5H]  f32 (this core's rows of evf)
         adjt [N, R]   f16 (adj[my rows, :]^T, host prep)
         woa  [F, F] f16, wosv [F, 2] f32
    out: whoa16 [R, F+1] f16 (hcat@Wo + ones col)
         svo    [R, 2]  f32 (s1o, s2o raw)
    """
    nc = bacc.Bacc("TRN2", target_bir_lowering=False, debug=False, num_devices=M)
    wha = nc.dram_tensor("wha", [N, H * HC], f16, kind="ExternalInput").ap()
    evf = nc.dram_tensor("evf", [N, 5 * H], f32, kind="ExternalInput").ap()
    # evt16 rows 0..7: f1 = exp(-0.8 s1) per head; rows 8..15: s1 - BIG
    # (host-transposed so one contiguous DMA stages every per-head row)
    evt16 = nc.dram_tensor("evt16", [32, R], f16, kind="ExternalInput").ap()
    # block-selector weights (host constant): bc[k, s*128+m] = (k==s),
    # zero-padded to K=32 to match the PE tile row granularity
    bcd = nc.dram_tensor("bcd", [32, 2 * H * 128], f16, kind="ExternalInput").ap()
    adjt = nc.dram_tensor("adjt", [N, R], f16, kind="ExternalInput").ap()
    woa = nc.dram_tensor("woa", [F, F], f16, kind="ExternalInput").ap()
    wosv = nc.dram_tensor("wosv", [F, 2], f32, kind="ExternalInput").ap()
    whoa16 = nc.dram_tensor("whoa16", [R, F + 1], f16, kind="ExternalOutput").ap()
    svo = nc.dram_tensor("svo", [R, 2], f32, kind="ExternalOutput").ap()

    B_HEADS = [h for h in range(H) if h not in A_HEADS]
    with tile.TileContext(nc) as tc:
        with (
            tc.tile_pool(name="sb", bufs=1) as sb,
            tc.tile_pool(name="work", bufs=2) as work,
            tc.tile_pool(name="pw", bufs=4) as pw,
        ):
            # --- prep FIRST so its small DMAs are not stuck behind the big
            # resident loads on the HWDGE queue ---
            evts = sb.tile([32, R], f16, tag="evts")
            nc.sync.dma_start(out=evts, in_=evt16)
            ident = sb.tile([128, 128], f32, tag="ident")
            make_identity(nc, ident)
            bigi = sb.tile([128, 128], f16, tag="bigi")
            nc.vector.tensor_scalar(bigi, ident, BIG, None, op0=OP.mult)
            # BC: block-selector weights; BC[k, s*128+m] = (k==s) so lhsT
            # slice s broadcasts evts row s across all 128 psum partitions
            BC = sb.tile([32, 2 * H * 128], f16, tag="BC")
            nc.sync.dma_start(out=BC, in_=bcd)

            # --- resident loads, grouped 3D-AP DMAs; small groups so the
            # jb-outer main loop can start on group 0 while the rest stream ---
            GB = 4                      # jb blocks per DMA group
            NG = JB // GB               # 8 groups
            adjt_g = adjt.rearrange("(g b p) r -> g p b r", b=GB, p=128)
            wha_g = wha.rearrange("(g b p) c -> g p b c", b=GB, p=128)
            evb = sb.tile([128, JB, 5 * H], f32, tag="evb")
            nc.sync.dma_start(
                out=evb, in_=evf.rearrange("(b p) c -> p b c", p=128)
            )
            adjtb, whab = [], []
            for g in range(NG):
                t = sb.tile([128, GB, R], f16, tag=f"adjtb{g}", name=f"adjtb{g}")
                nc.sync.dma_start(out=t, in_=adjt_g[g])
                adjtb.append(t)
                t = sb.tile([128, GB, H * HC], f16, tag=f"whab{g}", name=f"whab{g}")
                nc.sync.dma_start(out=t, in_=wha_g[g])
                whab.append(t)
            adjts = [adjtb[jb // GB][:, jb % GB, :] for jb in range(JB)]
            whas = [whab[jb // GB][:, jb % GB, :] for jb in range(JB)]
            evs = [evb[:, jb, :] for jb in range(JB)]

            # Who weights: only needed at the end, so issue their DMAs last
            woab = sb.tile([128, 4, F], f16, tag="woab")
            nc.sync.dma_start(out=woab, in_=woa.rearrange("(c p) f -> p c f", p=128))
            woas = [woab[:, fc, :] for fc in range(4)]
            wosv32 = work.tile([128, 4, 2], f32, tag="wosv32")
            nc.sync.dma_start(out=wosv32, in_=wosv.rearrange("(c p) t -> p c t", p=128))
            wosvb = sb.tile([128, 4, 2], f16, tag="wosvb")
            nc.scalar.activation(wosvb, wosv32, AF.Copy)
            wosvs = [wosvb[:, fc, :] for fc in range(4)]

            # f1 broadcast tiles via BC selector matmuls (K=16); skip heads
            # with no DVE/Pool-routed tiles (f1b would go unread)
            need_f1b = {h for h in range(H) for jb in range(JB)
                        if _k1_route(h, jb) != 'a'}
            f1bs = {}
            with tc.tile_pool(name="pp", bufs=2, space="PSUM") as pp:
                for h in sorted(need_f1b):
                    fp = pp.tile([128, R], f32, tag="fp")
                    nc.tensor.matmul(
                        fp, BC[:, h * 128:(h + 1) * 128], evts, start=True, stop=True
                    )
                    f1b = sb.tile([128, R], f16, tag=f"f1b{h}")
                    if h % 2 == 0:
                        nc.scalar.activation(f1b, fp, AF.Copy)
                    else:
                        nc.vector.tensor_copy(f1b, fp)
                    f1bs[h] = f1b

            hcats = [sb.tile([128, F], f16, tag=f"hcat{ic}", name=f"hcat{ic}") for ic in range(IC)]
            hTs = [sb.tile([128, R], f16, tag=f"hT{fc}", name=f"hT{fc}") for fc in range(4)]
            ident16 = sb.tile([128, 128], f16, tag="ident16")
            nc.scalar.activation(ident16, ident, AF.Copy)

            # --- main loop, jb-outer, 1-block software pipeline: p tiles for
            # block jb are built while PE consumes block jb-1, so PE's
            # in-order stream never stalls on the slowest elementwise engine ---
            with tc.tile_pool(name="ap", bufs=1, space="PSUM") as accp:
                # 6 banks: 4x B-head accumulators + 2x A-head accumulators.
                # B chains share a bank, and a start=True matmul resets
                # sibling slices in the same bank, so the bank is zeroed
                # once up front and every B matmul accumulates (start=False).
                accs = [
                    accp.tile([128, len(B_HEADS) * HC], f32, tag=f"acc{ic}", name=f"acc{ic}")
                    for ic in range(IC)
                ]
                for t in accs:
                    nc.vector.memset(t, 0.0)
                outTs = {
                    h: accp.tile([HC, R], f32, tag=f"outT{h}", name=f"outT{h}")
                    for h in A_HEADS
                }
                lp = tc.tile_pool(name="ep", bufs=2, space="PSUM")
                lp2 = tc.tile_pool(name="pw2", bufs=12)
                epp, pw2 = lp.__enter__(), lp2.__enter__()

                def make_p(h, jb, rt):
                    ev = evs[jb]
                    f2c, e2c = ev[:, H + h:H + h + 1], ev[:, h:h + 1]
                    if rt == 'a':
                        # e' = (s1[i]-BIG) + BIG*adjT: s1 row broadcast via
                        # the BC selector (row 8+h of evts), mask via BIG*I
                        eps = epp.tile([128, R], f32, tag="eps")
                        nc.tensor.matmul(
                            eps, BC[:, (H + h) * 128:(H + h + 1) * 128], evts,
                            start=True, stop=False,
                        )
                        nc.tensor.matmul(eps, bigi, adjts[jb], start=False, stop=True)
                        m = pw.tile([128, R], f16, tag="m")
                        nc.scalar.activation(
                            m, eps, AF.Prelu, alpha=ALPHA,
                            bias=evs[jb][:, 4 * H + h:4 * H + h + 1],
                        )
                        p = pw2.tile([128, R], f16, tag="pa")
                        nc.scalar.activation(p, m, AF.Exp)
                    else:
                        eng = nc.vector if rt == 'd' else nc.gpsimd
                        w = pw.tile([128, R], f16, tag="w" + rt)
                        eng.tensor_scalar(
                            w, f1bs[h], f2c, e2c, op0=OP.mult, op1=OP.max
                        )
                        p = pw2.tile([128, R], f16, tag="p" + rt)
                        eng.tensor_tensor(p, w, adjts[jb], op=OP.mult)
                    return p

                def accum(h, jb, p):
                    st, sp = (jb == 0), (jb == JB - 1)
                    wslice = whas[jb][:, h * HC:(h + 1) * HC]
                    if h in A_HEADS:
                        nc.tensor.matmul(outTs[h], wslice, p, start=st, stop=sp)
                    else:
                        hb = B_HEADS.index(h)
                        for ic in range(IC):
                            nc.tensor.matmul(
                                accs[ic][:, hb * HC:(hb + 1) * HC],
                                p[:, ic * 128:(ic + 1) * 128], wslice,
                                start=False, stop=sp, skip_group_check=True,
                            )

                def by_route(jb):
                    routed = [(h, _k1_route(h, jb)) for h in range(H)]
                    return [x for x in routed if x[1] == 'a'] + \
                           [x for x in routed if x[1] == 'd'] + \
                           [x for x in routed if x[1] == 'p']

                prev = None
                for jb in range(JB + 1):
                    if jb < JB:
                        cur = {h: make_p(h, jb, rt) for h, rt in by_route(jb)}
                    if prev is not None:
                        for h, rt in by_route(jb - 1):
                            accum(h, jb - 1, prev[h])
                    prev = cur
                lp2.__exit__(None, None, None)
                lp.__exit__(None, None, None)

                # --- finalize phase 1 (needs acc psum): per-row scale of
                # B-head outputs into f16 hcat, ELU of the B columns right
                # behind each chunk's scales, then drain outT to SBUF ---
                NB = len(B_HEADS) * NH
                for ic in range(IC):
                    r6 = sb.tile([128, len(B_HEADS)], f32, tag=f"r6_{ic}")
                    nc.vector.reciprocal(
                        r6, accs[ic].rearrange("p (h c) -> p h c", c=HC)[:, :, NH]
                    )
                    for hb, h in enumerate(B_HEADS):
                        dst = hcats[ic][:, h * NH:(h + 1) * NH]
                        src = accs[ic][:, hb * HC:hb * HC + NH]
                        if hb % 2 == 0:
                            nc.scalar.activation(
                                dst, src, AF.Copy, scale=r6[:, hb:hb + 1]
                            )
                        else:
                            nc.vector.tensor_scalar(
                                dst, src, r6[:, hb:hb + 1], None, op0=OP.mult
                            )
                    # ELU of B cols: elu(x) = min(exp(x) - 1, relu(x))
                    hb_slice = hcats[ic][:, 0:NB]
                    ex = work.tile([128, NB], f32, tag="ex")
                    nc.scalar.activation(ex, hb_slice, AF.Exp)
                    rl = work.tile([128, NB], f16, tag="rl")
                    nc.vector.tensor_scalar(rl, hb_slice, 0.0, None, op0=OP.max)
                    nc.vector.scalar_tensor_tensor(
                        hb_slice, ex, -1.0, rl, op0=OP.add, op1=OP.min
                    )
                ots = {}
                for h in A_HEADS:
                    ot = sb.tile([HC, R], f32, tag=f"ot{h}")
                    nc.vector.tensor_copy(ot, outTs[h])
                    ots[h] = ot

            # --- finalize phase 2 (acc banks freed): A-head transposes and
            # scales, bulk ELU per row chunk, hcatT, Who -- per-ic chains ---
            with (
                tc.tile_pool(name="fz", bufs=2, space="PSUM") as fzp,
                tc.tile_pool(name="fp2", bufs=2, space="PSUM") as fp2,
                tc.tile_pool(name="rr", bufs=4) as rr,
            ):
                for ic in range(IC):
                    wop = fp2.tile([128, F], f32, tag="wop")
                    svp = fp2.tile([128, 2], f32, tag="svp2")
                    # B columns are already ELU'd: transpose fc0..2 into hT
                    # and start the Who accumulation before the A chain lands
                    for fc in range(3):
                        tp2 = fzp.tile([128, 128], f16, tag="tp2")
                        nc.tensor.transpose(
                            tp2, hcats[ic][:, fc * 128:(fc + 1) * 128], ident16
                        )
                        dst = hTs[fc][:, ic * 128:(ic + 1) * 128]
                        if fc % 2 == 0:
                            nc.vector.tensor_copy(dst, tp2)
                        else:
                            nc.scalar.activation(dst, tp2, AF.Copy)
                        nc.tensor.matmul(
                            wop, dst, woas[fc], start=(fc == 0), stop=False
                        )
                        nc.tensor.matmul(
                            svp, dst, wosvs[fc], start=(fc == 0), stop=False
                        )
                    for hh, h in enumerate(A_HEADS):
                        tp = fzp.tile([128, HC], f32, tag="tp")
                        nc.tensor.transpose(
                            tp, ots[h][:, ic * 128:(ic + 1) * 128], ident[0:HC, 0:HC]
                        )
                        r = rr.tile([128, 1], f32, tag="r")
                        nc.vector.reciprocal(r, tp[:, NH:NH + 1])
                        dst = hcats[ic][:, h * NH:(h + 1) * NH]
                        if hh == 0:
                            nc.scalar.activation(
                                dst, tp[:, 0:NH], AF.Copy, scale=r
                            )
                        else:
                            nc.vector.tensor_scalar(
                                dst, tp[:, 0:NH], r, None, op0=OP.mult
                            )
                    # ELU of the A columns, then the last hT chunk
                    a3 = hcats[ic][:, 3 * 128:4 * 128]
                    ex3 = work.tile([128, 128], f32, tag="ex3")
                    nc.scalar.activation(ex3, a3, AF.Exp)
                    rl3 = rr.tile([128, 128], f16, tag="rl3")
                    nc.vector.tensor_scalar(rl3, a3, 0.0, None, op0=OP.max)
                    nc.vector.scalar_tensor_tensor(
                        a3, ex3, -1.0, rl3, op0=OP.add, op1=OP.min
                    )
                    tp2 = fzp.tile([128, 128], f16, tag="tp2")
                    nc.tensor.transpose(tp2, a3, ident16)
                    dst3 = hTs[3][:, ic * 128:(ic + 1) * 128]
                    nc.vector.tensor_copy(dst3, tp2)
                    nc.tensor.matmul(wop, dst3, woas[3], start=False, stop=True)
                    nc.tensor.matmul(svp, dst3, wosvs[3], start=False, stop=True)
                    wt = work.tile([128, F + 1], f16, tag="wt")
                    if ic % 2 == 0:
                        nc.scalar.activation(wt[:, 0:F], wop, AF.Copy)
                    else:
                        nc.vector.tensor_copy(wt[:, 0:F], wop)
                    nc.vector.memset(wt[:, F:F + 1], 1.0)
                    nc.sync.dma_start(
                        out=whoa16[ic * 128:(ic + 1) * 128, :], in_=wt
                    )
                    st = work.tile([128, 2], f32, tag="st")
                    nc.vector.tensor_copy(st, svp)
                    nc.sync.dma_start(out=svo[ic * 128:(ic + 1) * 128, :], in_=st)
    nc.compile()
    return nc


# ---------------------------------------------------------------- k2
def _build_k2():
    """Per-core: output-layer attention for this core's R rows, final ELU.

    in:  whoa [N, F+1] f16 (from k1), adjt [N, R] f16,
         ef [128, JB, 2] f32 (host: exp(s2o+bias) | exp(0.2 s2o + bias),
         blocked by key block), f1r [1, R] f16 (host: exp(-0.8 s1o) my rows)
    out: out [R, F] f32
    """
    nc = bacc.Bacc("TRN2", target_bir_lowering=False, debug=False, num_devices=M)
    whoa = nc.dram_tensor("whoa", [N, F + 1], f16, kind="ExternalInput").ap()
    ef = nc.dram_tensor("ef", [128, JB, 2], f32, kind="ExternalInput").ap()
    f1r = nc.dram_tensor("f1r", [1, R], f16, kind="ExternalInput").ap()
    adjt = nc.dram_tensor("adjt", [N, R], f16, kind="ExternalInput").ap()
    out = nc.dram_tensor("out", [R, F], f32, kind="ExternalOutput").ap()

    with tile.TileContext(nc) as tc:
        with (
            tc.tile_pool(name="sb", bufs=1) as sb,
            tc.tile_pool(name="work", bufs=4) as work,
            tc.tile_pool(name="pw", bufs=4) as pw,
        ):
            # --- prep first (small DMAs ahead of the big resident loads) ---
            efb = sb.tile([128, JB, 2], f32, tag="efb")
            nc.sync.dma_start(out=efb, in_=ef)
            f1rt = sb.tile([1, R], f16, tag="f1rt")
            nc.sync.dma_start(out=f1rt, in_=f1r)
            ones16 = sb.tile([1, 128], f16, tag="ones16")
            nc.vector.memset(ones16, 1.0)
            f1bo = sb.tile([128, R], f16, tag="f1bo")

            # --- resident loads, grouped 3D-AP DMAs ---
            GB = 4
            NG = JB // GB
            adjt_g = adjt.rearrange("(g b p) r -> g p b r", b=GB, p=128)
            whoa_g = whoa.rearrange("(g b p) c -> g p b c", b=GB, p=128)
            adjtb, whob = [], []
            for g in range(NG):
                t = sb.tile([128, GB, R], f16, tag=f"adjtb{g}", name=f"adjtb{g}")
                nc.sync.dma_start(out=t, in_=adjt_g[g])
                adjtb.append(t)
                t = sb.tile([128, GB, F + 1], f16, tag=f"whob{g}", name=f"whob{g}")
                nc.sync.dma_start(out=t, in_=whoa_g[g])
                whob.append(t)
            adjts = [adjtb[jb // GB][:, jb % GB, :] for jb in range(JB)]
            whos = [whob[jb // GB][:, jb % GB, :] for jb in range(JB)]

            with tc.tile_pool(name="pp0", bufs=1, space="PSUM") as pp0:
                fbp = pp0.tile([128, R], f32, tag="fbp")
                nc.tensor.matmul(fbp, ones16, f1rt, start=True, stop=True)
                nc.scalar.activation(f1bo, fbp, AF.Copy)

            # --- main loop, jb-outer with a 1-block stagger ---
            with (
                tc.tile_pool(name="ap", bufs=1, space="PSUM") as accp,
                tc.tile_pool(name="pp", bufs=1, space="PSUM") as ppp,
                tc.tile_pool(name="pw2", bufs=8) as pw2,
            ):
                accs = [accp.tile([128, F], f32, tag=f"acc{ic}", name=f"acc{ic}") for ic in range(IC)]
                rss = [ppp.tile([128, 1], f32, tag=f"rs{ic}", name=f"rs{ic}") for ic in range(IC)]

                def make_p2(jb):
                    eng = nc.gpsimd if jb % 5 == 4 else nc.vector
                    w = pw.tile([128, R], f16, tag="w")
                    eng.tensor_scalar(
                        w, f1bo, efb[:, jb, 1:2], efb[:, jb, 0:1],
                        op0=OP.mult, op1=OP.max,
                    )
                    p = pw2.tile([128, R], f16, tag="p")
                    eng.tensor_tensor(p, w, adjts[jb], op=OP.mult)
                    return p

                prev = None
                for jb in range(JB + 1):
                    if jb < JB:
                        cur = make_p2(jb)
                    if prev is not None:
                        pj = jb - 1
                        st, sp = (pj == 0), (pj == JB - 1)
                        for ic in range(IC):
                            nc.tensor.matmul(
                                accs[ic], prev[:, ic * 128:(ic + 1) * 128],
                                whos[pj][:, 0:F], start=st, stop=sp,
                            )
                            nc.tensor.matmul(
                                rss[ic], prev[:, ic * 128:(ic + 1) * 128],
                                whos[pj][:, F:F + 1], start=st, stop=sp,
                            )
                    prev = cur

                # scaled ELU straight off psum:
                # elu(r*x) = min(exp(r*x) - 1, relu(r*x))
                for ic in range(IC):
                    r = work.tile([128, 1], f32, tag="r")
                    nc.vector.reciprocal(r, rss[ic])
                    ex = work.tile([128, F], f32, tag="ex")
                    nc.scalar.activation(ex, accs[ic], AF.Exp, scale=r)
                    rl = work.tile([128, F], f32, tag="rl")
                    nc.vector.tensor_scalar(
                        rl, accs[ic], r, 0.0, op0=OP.mult, op1=OP.max
                    )
                    ot = work.tile([128, F], f32, tag="ot")
                    nc.vector.scalar_tensor_tensor(
                        ot, ex, -1.0, rl, op0=OP.add, op1=OP.min
                    )
                    nc.sync.dma_start(out=out[ic * 128:(ic + 1) * 128, :], in_=ot)
    nc.compile()
    return nc


def _get(name):
    if name not in _CACHE:
        _CACHE[name] = {"k0": _build_k0, "k1": _build_k1, "k2": _build_k2}[name]()
    return _CACHE[name]


# ---------------------------------------------------------------- host
def kernel(x, left, adj, Ws, a1, a2, Wo, ao1, ao2):
    x = np.asarray(x, np.float32)
    adj = np.asarray(adj, np.float32)
    Ws = np.asarray(Ws, np.float32)
    a1 = np.asarray(a1, np.float32)
    a2 = np.asarray(a2, np.float32)
    Wo = np.asarray(Wo, np.float32)
    ao1 = np.asarray(ao1, np.float32)
    ao2 = np.asarray(ao2, np.float32)

    # host-side layout prep (no significant FLOPs)
    ws_all = np.ascontiguousarray(Ws.transpose(1, 0, 2).reshape(F, F))
    ws1 = np.einsum("hkf,hf->kh", Ws, a1)   # [F, H]  tiny matvecs
    ws2 = np.einsum("hkf,hf->kh", Ws, a2)
    wsa16 = ws_all.astype(np.float16)
    wsv16 = np.ascontiguousarray(
        np.concatenate([ws1, ws2], axis=1)
    ).astype(np.float16)
    woa = np.ascontiguousarray(Wo).astype(np.float16)
    wosv = np.ascontiguousarray(
        np.stack([Wo @ ao1, Wo @ ao2], axis=1), dtype=np.float32
    )
    adj16 = adj.astype(np.float16)  # exact: adj is a 0/1 mask
    adjt_c = [
        np.ascontiguousarray(adj16[c * R:(c + 1) * R].T) for c in range(M)
    ]
    xt_c = [np.ascontiguousarray(x[c * R:(c + 1) * R].T) for c in range(M)]

    cores = list(range(M))

    k0 = _get("k0")
    res0 = _run(
        k0,
        [
            {"xin": np.concatenate(
                [xt_c[c].astype(np.float16), wsa16, wsv16], axis=1
            )}
            for c in cores
        ],
        cores,
    )
    wha = np.concatenate([res0.results[c]["wha16"] for c in cores], axis=0)
    evf = np.concatenate([res0.results[c]["ev"] for c in cores], axis=0)

    def evt16_for(c):
        ev = evf[c * R:(c + 1) * R]
        out = np.zeros((32, R), np.float16)
        out[0:H] = ev[:, 2 * H:3 * H].T
        out[H:2 * H] = ev[:, 3 * H:4 * H].T - BIG
        return out

    bcd = np.zeros((32, 2 * H * 128), np.float16)
    bcd[0:2 * H] = np.repeat(np.eye(2 * H, dtype=np.float16), 128, axis=1)
    k1 = _get("k1")
    res1 = _run(
        k1,
        [
            {
                "wha": wha,
                "evf": evf,
                "evt16": evt16_for(c),
                "bcd": bcd,
                "adjt": adjt_c[c],
                "woa": woa,
                "wosv": wosv,
            }
            for c in cores
        ],
        cores,
    )
    whoa = np.concatenate([res1.results[c]["whoa16"] for c in cores], axis=0)
    svof = np.concatenate([res1.results[c]["svo"] for c in cores], axis=0)

    # k2 scalar prep (tiny): shifted exponentials of the output-layer scores
    s1o, s2o = svof[:, 0].astype(np.float64), svof[:, 1].astype(np.float64)
    bias = 9.0 - s2o.max()
    ef = np.empty((128, JB, 2), np.float32)
    ef[:, :, 0] = np.exp(s2o + bias).reshape(JB, 128).T
    ef[:, :, 1] = np.exp(0.2 * s2o + bias).reshape(JB, 128).T
    f1o = np.exp(-0.8 * s1o).astype(np.float16)

    k2 = _get("k2")
    res2 = _run(
        k2,
        [
            {
                "whoa": whoa,
                "ef": ef,
                "f1r": f1o[c * R:(c + 1) * R].reshape(1, R),
                "adjt": adjt_c[c],
            }
            for c in cores
        ],
        cores,
    )
    return np.concatenate([res2.results[c]["out"] for c in cores], axis=0)



# revision 74
# speedup vs baseline: 1.0068x; 1.0068x over previous
"""GAT (graph attention network) forward pass on 8 Trainium2 NeuronCores.

Problem: nn_GAT - N=4096 nodes, F=512 features, H=8 heads, 1% dense adjacency.
    heads:  Wh = x @ Ws[h]; e = lrelu(s1[i]+s2[j]); att = masked softmax; elu(att @ Wh)
    out layer: same attention structure on hcat @ Wo, then elu.

Strategy (row-sharded across 8 cores, 3 launches):
  k0: each core computes Wh (all heads, one fused f16 matmul per 128-row
      chunk) + raw/exp'd score vectors for its 512 nodes; host gathers.
  k1: each core runs 8-head masked-softmax attention for its 512 query rows.
      Key identities: exp(lrelu(e)) = max(exp(e), exp(0.2e)); exp(e) factors
      rank-1 as exp(s1)[i]*exp(s2)[j]; and softmax tolerates any per-query-row
      scale, so scaling row i by exp(-s1[i]) turns both branches into
      per-partition scalars -- the NxN tiles need NO transcendentals:
        w = (f1b * f2[j]) max e2[j]   (one 2-op tensor_scalar, DVE 4x mode)
        p = w * adjT                  (mask multiply, fp16)
      Tiles are routed across three engine pipelines (see _k1_route), the
      jb-outer loop is software-pipelined one block deep, and six heads
      accumulate in out[i,c] orientation (65-col matmuls) with the psum
      banks zeroed up front because start=True resets bank-sharing siblings.
      The softmax denominator comes free as a ones-column in the value
      matrix; ELU uses elu(x) = min(exp(x)-1, relu(x)).
      Also computes hcat @ Wo (+ output-layer score vectors) for its rows.
  k2: output-layer attention for the core's 512 rows (same masked-softmax
      structure, one head of 513 value columns); final ELU.

adj is passed from host as a pre-transposed fp16 (exact for a 0/1 mask) slice
per core; x/weights arrive pre-transposed f16 (pure layout prep, no FLOPs).
"""

import sys

for _p in ("/opt/trn_rl_repo",):
    if _p not in sys.path:
        sys.path.insert(0, _p)

import numpy as np

import concourse.bass as bass
import concourse.tile as tile
from concourse import bacc, mybir
from concourse.bass_utils import run_bass_kernel_spmd
from concourse.masks import make_identity

N, F, H, NH = 4096, 512, 8, 64
M = 8            # cores
R = N // M       # 512 query rows per core
JB = N // 128    # 32 key blocks
IC = R // 128    # 4 query-row chunks per core
HC = NH + 1      # 65 value cols per head (64 + ones col for row sums)
ALPHA = 0.2     # leaky relu slope
BIG = 200.0     # additive mask; 0.2*BIG=40 so masked exp underflows to 0
f32 = mybir.dt.float32
f16 = mybir.dt.float16
OP = mybir.AluOpType
AF = mybir.ActivationFunctionType

_CACHE = {}


def _run(nc, in_maps, core_ids, tries=3):
    """run_bass_kernel_spmd with retry: the axon-tunneled devices
    occasionally report NRT_EXEC_UNIT_UNRECOVERABLE transiently."""
    import time as _time

    for attempt in range(tries):
        try:
            return run_bass_kernel_spmd(nc, in_maps, core_ids=core_ids)
        except Exception:
            if attempt == tries - 1:
                raise
            _time.sleep(5.0)


# ---------------------------------------------------------------- k0
def _build_k0():
    """Per-core: Wh projection for this core's R nodes, all heads fused.

    in:  xin [F, R+F+2H] f16 ([xT | Ws stacked | ws1 ws2 cols], host concat)
    out: wha16 [R, H*HC] f16 (per head: 64 value cols + ones col)
         ev [R, 5H] f32 (exp(s2) | exp(.2 s2) | exp(-.8 s1) | s1 | s2 per head)
    """
    XW = R + F + 2 * H
    nc = bacc.Bacc("TRN2", target_bir_lowering=False, debug=False, num_devices=M)
    # xin = [xT | Ws stacked | ws1 ws2 cols] concatenated on host: one DMA
    # generation per 128-row chunk covers data and weights together
    xin = nc.dram_tensor("xin", [F, XW], f16, kind="ExternalInput").ap()
    wha16 = nc.dram_tensor("wha16", [R, H * HC], f16, kind="ExternalOutput").ap()
    ev = nc.dram_tensor("ev", [R, 5 * H], f32, kind="ExternalOutput").ap()

    with tile.TileContext(nc) as tc:
        with (
            tc.tile_pool(name="sb", bufs=1) as sb,
            tc.tile_pool(name="ps", bufs=1, space="PSUM") as ps,
            tc.tile_pool(name="ob", bufs=4) as ob,
        ):
            xin_v = xin.rearrange("(c p) w -> p c w", p=128)
            xwb = []
            for fc in range(4):
                t = sb.tile([128, XW], f16, tag=f"xwb{fc}", name=f"xwb{fc}")
                nc.sync.dma_start(out=t, in_=xin_v[:, fc, :])
                xwb.append(t)
            evall = sb.tile([128, 4, 5 * H], f32, tag="evall")
            # fc-outer accumulation: all 8 psum chains live, so PE starts
            # after the first fc chunk lands and never waits again
            whps = [ps.tile([128, F], f32, tag=f"whp{nb}", name=f"whp{nb}")
                    for nb in range(IC)]
            svps = [ps.tile([128, 2 * H], f32, tag=f"svp{nb}", name=f"svp{nb}")
                    for nb in range(IC)]
            for fc in range(4):
                xt = xwb[fc][:, 0:R]
                ws = xwb[fc][:, R:R + F]
                wv = xwb[fc][:, R + F:XW]
                st, sp = (fc == 0), (fc == 3)
                for nb in range(IC):
                    nc.tensor.matmul(
                        whps[nb], xt[:, nb * 128:(nb + 1) * 128], ws,
                        start=st, stop=sp,
                    )
                    nc.tensor.matmul(
                        svps[nb], xt[:, nb * 128:(nb + 1) * 128], wv,
                        start=st, stop=sp,
                    )
                    if sp:
                        # finalize nb as soon as its chain stops
                        whp, svp = whps[nb], svps[nb]
                        wt = ob.tile([128, H * HC], f16, tag="wt")
                        wt_v = wt.rearrange("p (h c) -> p h c", c=HC)
                        nc.vector.tensor_copy(
                            wt_v[:, :, 0:NH],
                            whp.rearrange("p (h c) -> p h c", c=NH),
                        )
                        nc.vector.memset(wt_v[:, :, NH], 1.0)
                        nc.sync.dma_start(
                            out=wha16[nb * 128:(nb + 1) * 128, :], in_=wt
                        )
                        evt = evall[:, nb, :]
                        # sv cols: 0..H-1 = s1, H..2H-1 = s2 per head
                        nc.scalar.activation(evt[:, 0:H], svp[:, H:2 * H], AF.Exp)
                        nc.scalar.activation(
                            evt[:, H:2 * H], svp[:, H:2 * H], AF.Exp, scale=0.2
                        )
                        nc.scalar.activation(
                            evt[:, 2 * H:3 * H], svp[:, 0:H], AF.Exp, scale=-0.8
                        )
                        nc.vector.tensor_copy(evt[:, 3 * H:5 * H], svp[:, 0:2 * H])
            nc.sync.dma_start(
                out=ev.rearrange("(b p) c -> p b c", p=128), in_=evall
            )
    nc.compile()
    return nc


# ---------------------------------------------------------------- k1
# Tile routing: per (head, key-block) the masked-exp tile is built on one of
# three engine pipelines, ratios chosen from the TimelineSim cost model
# (DVE pair ~499ns, Pool pair ~1917ns, ACT additive-mask route ~1224ns):
#   'd' (DVE):  w = (f1b*f2) max e2 [TS 4x], p = w*adjT [TT 2x]
#   'p' (Pool): same two ops on GPSIMD
#   'a' (ACT):  e' = s1-BIG + BIG*adjT [PE->PSUM], p = exp(prelu(e'+s2)) [ACT]
# Heads 0-5 accumulate orientation-B (out[i,c]; 65-col matmuls, 4 per tile);
# heads 6-7 accumulate orientation-A (outT[c,i]; one 512-col matmul) so the
# PSUM budget is exactly 8 banks: 4x acc_ic[128,390] + 2x outT[65,512] +
# 2x eps[128,512] double-buffered.
A_HEADS = (6, 7)


def _k1_route(h, jb):
    # NOTE: the 'a' route produces p at a different per-row scale than
    # 'd'/'p' (which fold in exp(-s1[i])); softmax cancels a per-row scale
    # only if every key-block of a head agrees, so a head is either all-'a'
    # or a d/p mix -- never both.
    if h in A_HEADS:
        return 'a'
    return 'p' if (h * JB + jb) % 14 < 3 else 'd'


def _build_k1():
    """Per-core: 8-head attention for this core's R query rows + Who projection.

    in:  wha  [N, H*HC] f16 (full, from k0)
         evf  [N, 5H]  f32 (full)
         evmy [R, 5H]  f32 (this core's rows of evf)
         adjt [N, R]   f16 (adj[my rows, :]^T, host prep)
         woa  [F, F] f16, wosv [F, 2] f32
    out: whoa16 [R, F+1] f16 (hcat@Wo + ones col)
         svo    [R, 2]  f32 (s1o, s2o raw)
    """
    nc = bacc.Bacc("TRN2", target_bir_lowering=False, debug=False, num_devices=M)
    wha = nc.dram_tensor("wha", [N, H * HC], f16, kind="ExternalInput").ap()
    evf = nc.dram_tensor("evf", [N, 5 * H], f32, kind="ExternalInput").ap()
    # evt16 rows 0..7: f1 = exp(-0.8 s1) per head; rows 8..15: s1 - BIG
    # (host-transposed so one contiguous DMA stages every per-head row)
    evt16 = nc.dram_tensor("evt16", [32, R], f16, kind="ExternalInput").ap()
    # block-selector weights (host constant): bc[k, s*128+m] = (k==s),
    # zero-padded to K=32 to match the PE tile row granularity
    bcd = nc.dram_tensor("bcd", [32, 2 * H * 128], f16, kind="ExternalInput").ap()
    adjt = nc.dram_tensor("adjt", [N, R], f16, kind="ExternalInput").ap()
    woa = nc.dram_tensor("woa", [F, F], f16, kind="ExternalInput").ap()
    wosv = nc.dram_tensor("wosv", [F, 2], f32, kind="ExternalInput").ap()
    whoa16 = nc.dram_tensor("whoa16", [R, F + 1], f16, kind="ExternalOutput").ap()
    svo = nc.dram_tensor("svo", [R, 2], f32, kind="ExternalOutput").ap()

    B_HEADS = [h for h in range(H) if h not in A_HEADS]
    with tile.TileContext(nc) as tc:
        with (
            tc.tile_pool(name="sb", bufs=1) as sb,
            tc.tile_pool(name="work", bufs=2) as work,
            tc.tile_pool(name="pw", bufs=4) as pw,
        ):
            # --- prep FIRST so its small DMAs are not stuck behind the big
            # resident loads on the HWDGE queue ---
            evts = sb.tile([32, R], f16, tag="evts")
            nc.sync.dma_start(out=evts, in_=evt16)
            ident = sb.tile([128, 128], f32, tag="ident")
            make_identity(nc, ident)
            bigi = sb.tile([128, 128], f16, tag="bigi")
            nc.vector.tensor_scalar(bigi, ident, BIG, None, op0=OP.mult)
            # BC: block-selector weights; BC[k, s*128+m] = (k==s) so lhsT
            # slice s broadcasts evts row s across all 128 psum partitions
            BC = sb.tile([32, 2 * H * 128], f16, tag="BC")
            nc.sync.dma_start(out=BC, in_=bcd)

            # --- resident loads, grouped 3D-AP DMAs; small groups so the
            # jb-outer main loop can start on group 0 while the rest stream ---
            GB = 4                      # jb blocks per DMA group
            NG = JB // GB               # 8 groups
            adjt_g = adjt.rearrange("(g b p) r -> g p b r", b=GB, p=128)
            wha_g = wha.rearrange("(g b p) c -> g p b c", b=GB, p=128)
            evb = sb.tile([128, JB, 5 * H], f32, tag="evb")
            nc.sync.dma_start(
                out=evb, in_=evf.rearrange("(b p) c -> p b c", p=128)
            )
            adjtb, whab = [], []
            for g in range(NG):
                t = sb.tile([128, GB, R], f16, tag=f"adjtb{g}", name=f"adjtb{g}")
                nc.sync.dma_start(out=t, in_=adjt_g[g])
                adjtb.append(t)
                t = sb.tile([128, GB, H * HC], f16, tag=f"whab{g}", name=f"whab{g}")
                nc.sync.dma_start(out=t, in_=wha_g[g])
                whab.append(t)
            adjts = [adjtb[jb // GB][:, jb % GB, :] for jb in range(JB)]
            whas = [whab[jb // GB][:, jb % GB, :] for jb in range(JB)]
            evs = [evb[:, jb, :] for jb in range(JB)]

            # Who weights: only needed at the end, so issue their DMAs last
            woab = sb.tile([128, 4, F], f16, tag="woab")
            nc.sync.dma_start(out=woab, in_=woa.rearrange("(c p) f -> p c f", p=128))
            woas = [woab[:, fc, :] for fc in range(4)]
            wosv32 = work.tile([128, 4, 2], f32, tag="wosv32")
            nc.sync.dma_start(out=wosv32, in_=wosv.rearrange("(c p) t -> p c t", p=128))
            wosvb = sb.tile([128, 4, 2], f16, tag="wosvb")
            nc.scalar.activation(wosvb, wosv32, AF.Copy)
            wosvs = [wosvb[:, fc, :] for fc in range(4)]

            # f1 broadcast tiles via BC selector matmuls (K=16); skip heads
            # with no DVE/Pool-routed tiles (f1b would go unread)
            need_f1b = {h for h in range(H) for jb in range(JB)
                        if _k1_route(h, jb) != 'a'}
            f1bs = {}
            with tc.tile_pool(name="pp", bufs=2, space="PSUM") as pp:
                for h in sorted(need_f1b):
                    fp = pp.tile([128, R], f32, tag="fp")
                    nc.tensor.matmul(
                        fp, BC[:, h * 128:(h + 1) * 128], evts, start=True, stop=True
                    )
                    f1b = sb.tile([128, R], f16, tag=f"f1b{h}")
                    if h % 2 == 0:
                        nc.scalar.activation(f1b, fp, AF.Copy)
                    else:
                        nc.vector.tensor_copy(f1b, fp)
                    f1bs[h] = f1b

            hcats = [sb.tile([128, F], f16, tag=f"hcat{ic}", name=f"hcat{ic}") for ic in range(IC)]
            hTs = [sb.tile([128, R], f16, tag=f"hT{fc}", name=f"hT{fc}") for fc in range(4)]
            ident16 = sb.tile([128, 128], f16, tag="ident16")
            nc.scalar.activation(ident16, ident, AF.Copy)

            # --- main loop, jb-outer, 1-block software pipeline: p tiles for
            # block jb are built while PE consumes block jb-1, so PE's
            # in-order stream never stalls on the slowest elementwise engine ---
            with tc.tile_pool(name="ap", bufs=1, space="PSUM") as accp:
                # 6 banks: 4x B-head accumulators + 2x A-head accumulators.
                # B chains share a bank, and a start=True matmul resets
                # sibling slices in the same bank, so the bank is zeroed
                # once up front and every B matmul accumulates (start=False).
                accs = [
                    accp.tile([128, len(B_HEADS) * HC], f32, tag=f"acc{ic}", name=f"acc{ic}")
                    for ic in range(IC)
                ]
                for t in accs:
                    nc.vector.memset(t, 0.0)
                outTs = {
                    h: accp.tile([HC, R], f32, tag=f"outT{h}", name=f"outT{h}")
                    for h in A_HEADS
                }
                lp = tc.tile_pool(name="ep", bufs=2, space="PSUM")
                lp2 = tc.tile_pool(name="pw2", bufs=12)
                epp, pw2 = lp.__enter__(), lp2.__enter__()

                def make_p(h, jb, rt):
                    ev = evs[jb]
                    f2c, e2c = ev[:, H + h:H + h + 1], ev[:, h:h + 1]
                    if rt == 'a':
                        # e' = (s1[i]-BIG) + BIG*adjT: s1 row broadcast via
                        # the BC selector (row 8+h of evts), mask via BIG*I
                        eps = epp.tile([128, R], f32, tag="eps")
                        nc.tensor.matmul(
                            eps, BC[:, (H + h) * 128:(H + h + 1) * 128], evts,
                            start=True, stop=False,
                        )
                        nc.tensor.matmul(eps, bigi, adjts[jb], start=False, stop=True)
                        m = pw.tile([128, R], f16, tag="m")
                        nc.scalar.activation(
                            m, eps, AF.Prelu, alpha=ALPHA,
                            bias=evs[jb][:, 4 * H + h:4 * H + h + 1],
                        )
                        p = pw2.tile([128, R], f16, tag="pa")
                        nc.scalar.activation(p, m, AF.Exp)
                    else:
                        eng = nc.vector if rt == 'd' else nc.gpsimd
                        w = pw.tile([128, R], f16, tag="w" + rt)
                        eng.tensor_scalar(
                            w, f1bs[h], f2c, e2c, op0=OP.mult, op1=OP.max
                        )
                        p = pw2.tile([128, R], f16, tag="p" + rt)
                        eng.tensor_tensor(p, w, adjts[jb], op=OP.mult)
                    return p

                def accum(h, jb, p):
                    st, sp = (jb == 0), (jb == JB - 1)
                    wslice = whas[jb][:, h * HC:(h + 1) * HC]
                    if h in A_HEADS:
                        nc.tensor.matmul(outTs[h], wslice, p, start=st, stop=sp)
                    else:
                        hb = B_HEADS.index(h)
                        for ic in range(IC):
                            nc.tensor.matmul(
                                accs[ic][:, hb * HC:(hb + 1) * HC],
                                p[:, ic * 128:(ic + 1) * 128], wslice,
                                start=False, stop=sp, skip_group_check=True,
                            )

                def by_route(jb):
                    routed = [(h, _k1_route(h, jb)) for h in range(H)]
                    return [x for x in routed if x[1] == 'a'] + \
                           [x for x in routed if x[1] == 'd'] + \
                           [x for x in routed if x[1] == 'p']

                prev = None
                for jb in range(JB + 1):
                    if jb < JB:
                        cur = {h: make_p(h, jb, rt) for h, rt in by_route(jb)}
                    if prev is not None:
                        for h, rt in by_route(jb - 1):
                            accum(h, jb - 1, prev[h])
                    prev = cur
                lp2.__exit__(None, None, None)
                lp.__exit__(None, None, None)

                # --- finalize phase 1 (needs acc psum): per-row scale of
                # B-head outputs into f16 hcat, ELU of the B columns right
                # behind each chunk's scales, then drain outT to SBUF ---
                NB = len(B_HEADS) * NH
                for ic in range(IC):
                    r6 = sb.tile([128, len(B_HEADS)], f32, tag=f"r6_{ic}")
                    nc.vector.reciprocal(
                        r6, accs[ic].rearrange("p (h c) -> p h c", c=HC)[:, :, NH]
                    )
                    for hb, h in enumerate(B_HEADS):
                        dst = hcats[ic][:, h * NH:(h + 1) * NH]
                        src = accs[ic][:, hb * HC:hb * HC + NH]
                        if hb % 2 == 0:
                            nc.scalar.activation(
                                dst, src, AF.Copy, scale=r6[:, hb:hb + 1]
                            )
                        else:
                            nc.vector.tensor_scalar(
                                dst, src, r6[:, hb:hb + 1], None, op0=OP.mult
                            )
                    # ELU of B cols: elu(x) = min(exp(x) - 1, relu(x))
                    hb_slice = hcats[ic][:, 0:NB]
                    ex = work.tile([128, NB], f32, tag="ex")
                    nc.scalar.activation(ex, hb_slice, AF.Exp)
                    rl = work.tile([128, NB], f16, tag="rl")
                    nc.vector.tensor_scalar(rl, hb_slice, 0.0, None, op0=OP.max)
                    nc.vector.scalar_tensor_tensor(
                        hb_slice, ex, -1.0, rl, op0=OP.add, op1=OP.min
                    )
                ots = {}
                for h in A_HEADS:
                    ot = sb.tile([HC, R], f32, tag=f"ot{h}")
                    nc.vector.tensor_copy(ot, outTs[h])
                    ots[h] = ot

            # --- finalize phase 2 (acc banks freed): A-head transposes and
            # scales, bulk ELU per row chunk, hcatT, Who -- per-ic chains ---
            with (
                tc.tile_pool(name="fz", bufs=3, space="PSUM") as fzp,
                tc.tile_pool(name="fp2", bufs=1, space="PSUM") as fp2,
                tc.tile_pool(name="rr", bufs=4) as rr,
            ):
                for ic in range(IC):
                    wop = fp2.tile([128, F], f32, tag="wop")
                    svp = fp2.tile([128, 2], f32, tag="svp2")
                    # B columns are already ELU'd: transpose fc0..2 into hT
                    # and start the Who accumulation before the A chain lands
                    for fc in range(3):
                        tp2 = fzp.tile([128, 128], f16, tag="tp2")
                        nc.tensor.transpose(
                            tp2, hcats[ic][:, fc * 128:(fc + 1) * 128], ident16
                        )
                        dst = hTs[fc][:, ic * 128:(ic + 1) * 128]
                        if fc % 2 == 0:
                            nc.vector.tensor_copy(dst, tp2)
                        else:
                            nc.scalar.activation(dst, tp2, AF.Copy)
                        nc.tensor.matmul(
                            wop, dst, woas[fc], start=(fc == 0), stop=False
                        )
                        nc.tensor.matmul(
                            svp, dst, wosvs[fc], start=(fc == 0), stop=False
                        )
                    for hh, h in enumerate(A_HEADS):
                        tp = fzp.tile([128, HC], f32, tag="tp")
                        nc.tensor.transpose(
                            tp, ots[h][:, ic * 128:(ic + 1) * 128], ident[0:HC, 0:HC]
                        )
                        r = rr.tile([128, 1], f32, tag="r")
                        nc.vector.reciprocal(r, tp[:, NH:NH + 1])
                        dst = hcats[ic][:, h * NH:(h + 1) * NH]
                        if hh == 0:
                            nc.scalar.activation(
                                dst, tp[:, 0:NH], AF.Copy, scale=r
                            )
                        else:
                            nc.vector.tensor_scalar(
                                dst, tp[:, 0:NH], r, None, op0=OP.mult
                            )
                    # ELU of the A columns, then the last hT chunk
                    a3 = hcats[ic][:, 3 * 128:4 * 128]
                    ex3 = work.tile([128, 128], f32, tag="ex3")
                    nc.scalar.activation(ex3, a3, AF.Exp)
                    rl3 = rr.tile([128, 128], f16, tag="rl3")
                    nc.vector.tensor_scalar(rl3, a3, 0.0, None, op0=OP.max)
                    nc.vector.scalar_tensor_tensor(
                        a3, ex3, -1.0, rl3, op0=OP.add, op1=OP.min
                    )
                    tp2 = fzp.tile([128, 128], f16, tag="tp2")
                    nc.tensor.transpose(tp2, a3, ident16)
                    dst3 = hTs[3][:, ic * 128:(ic + 1) * 128]
                    nc.vector.tensor_copy(dst3, tp2)
                    nc.tensor.matmul(wop, dst3, woas[3], start=False, stop=True)
                    nc.tensor.matmul(svp, dst3, wosvs[3], start=False, stop=True)
                    wt = work.tile([128, F + 1], f16, tag="wt")
                    if ic % 2 == 0:
                        nc.scalar.activation(wt[:, 0:F], wop, AF.Copy)
                    else:
                        nc.vector.tensor_copy(wt[:, 0:F], wop)
                    nc.vector.memset(wt[:, F:F + 1], 1.0)
                    nc.sync.dma_start(
                        out=whoa16[ic * 128:(ic + 1) * 128, :], in_=wt
                    )
                    st = work.tile([128, 2], f32, tag="st")
                    nc.vector.tensor_copy(st, svp)
                    nc.sync.dma_start(out=svo[ic * 128:(ic + 1) * 128, :], in_=st)
    nc.compile()
    return nc


# ---------------------------------------------------------------- k2
def _build_k2():
    """Per-core: output-layer attention for this core's R rows, final ELU.

    in:  whoa [N, F+1] f16 (from k1), adjt [N, R] f16,
         ef [128, JB, 2] f32 (host: exp(s2o+bias) | exp(0.2 s2o + bias),
         blocked by key block), f1r [1, R] f16 (host: exp(-0.8 s1o) my rows)
    out: out [R, F] f32
    """
    nc = bacc.Bacc("TRN2", target_bir_lowering=False, debug=False, num_devices=M)
    whoa = nc.dram_tensor("whoa", [N, F + 1], f16, kind="ExternalInput").ap()
    ef = nc.dram_tensor("ef", [128, JB, 2], f32, kind="ExternalInput").ap()
    f1r = nc.dram_tensor("f1r", [1, R], f16, kind="ExternalInput").ap()
    adjt = nc.dram_tensor("adjt", [N, R], f16, kind="ExternalInput").ap()
    out = nc.dram_tensor("out", [R, F], f32, kind="ExternalOutput").ap()

    with tile.TileContext(nc) as tc:
        with (
            tc.tile_pool(name="sb", bufs=1) as sb,
            tc.tile_pool(name="work", bufs=4) as work,
            tc.tile_pool(name="pw", bufs=4) as pw,
        ):
            # --- prep first (small DMAs ahead of the big resident loads) ---
            efb = sb.tile([128, JB, 2], f32, tag="efb")
            nc.sync.dma_start(out=efb, in_=ef)
            f1rt = sb.tile([1, R], f16, tag="f1rt")
            nc.sync.dma_start(out=f1rt, in_=f1r)
            ones16 = sb.tile([1, 128], f16, tag="ones16")
            nc.vector.memset(ones16, 1.0)
            f1bo = sb.tile([128, R], f16, tag="f1bo")

            # --- resident loads, grouped 3D-AP DMAs ---
            GB = 4
            NG = JB // GB
            adjt_g = adjt.rearrange("(g b p) r -> g p b r", b=GB, p=128)
            whoa_g = whoa.rearrange("(g b p) c -> g p b c", b=GB, p=128)
            adjtb, whob = [], []
            for g in range(NG):
                t = sb.tile([128, GB, R], f16, tag=f"adjtb{g}", name=f"adjtb{g}")
                nc.sync.dma_start(out=t, in_=adjt_g[g])
                adjtb.append(t)
                t = sb.tile([128, GB, F + 1], f16, tag=f"whob{g}", name=f"whob{g}")
                nc.sync.dma_start(out=t, in_=whoa_g[g])
                whob.append(t)
            adjts = [adjtb[jb // GB][:, jb % GB, :] for jb in range(JB)]
            whos = [whob[jb // GB][:, jb % GB, :] for jb in range(JB)]

            with tc.tile_pool(name="pp0", bufs=1, space="PSUM") as pp0:
                fbp = pp0.tile([128, R], f32, tag="fbp")
                nc.tensor.matmul(fbp, ones16, f1rt, start=True, stop=True)
                nc.scalar.activation(f1bo, fbp, AF.Copy)

            # --- main loop, jb-outer with a 1-block stagger ---
            with (
                tc.tile_pool(name="ap", bufs=1, space="PSUM") as accp,
                tc.tile_pool(name="pp", bufs=1, space="PSUM") as ppp,
                tc.tile_pool(name="pw2", bufs=8) as pw2,
            ):
                accs = [accp.tile([128, F], f32, tag=f"acc{ic}", name=f"acc{ic}") for ic in range(IC)]
                rss = [ppp.tile([128, 1], f32, tag=f"rs{ic}", name=f"rs{ic}") for ic in range(IC)]

                def make_p2(jb):
                    eng = nc.gpsimd if jb % 5 == 4 else nc.vector
                    w = pw.tile([128, R], f16, tag="w")
                    eng.tensor_scalar(
                        w, f1bo, efb[:, jb, 1:2], efb[:, jb, 0:1],
                        op0=OP.mult, op1=OP.max,
                    )
                    p = pw2.tile([128, R], f16, tag="p")
                    eng.tensor_tensor(p, w, adjts[jb], op=OP.mult)
                    return p

                prev = None
                for jb in range(JB + 1):
                    if jb < JB:
                        cur = make_p2(jb)
                    if prev is not None:
                        pj = jb - 1
                        st, sp = (pj == 0), (pj == JB - 1)
                        for ic in range(IC):
                            nc.tensor.matmul(
                                accs[ic], prev[:, ic * 128:(ic + 1) * 128],
                                whos[pj][:, 0:F], start=st, stop=sp,
                            )
                            nc.tensor.matmul(
                                rss[ic], prev[:, ic * 128:(ic + 1) * 128],
                                whos[pj][:, F:F + 1], start=st, stop=sp,
                            )
                    prev = cur

                # scaled ELU straight off psum:
                # elu(r*x) = min(exp(r*x) - 1, relu(r*x))
                for ic in range(IC):
                    r = work.tile([128, 1], f32, tag="r")
                    nc.vector.reciprocal(r, rss[ic])
                    ex = work.tile([128, F], f32, tag="ex")
                    nc.scalar.activation(ex, accs[ic], AF.Exp, scale=r)
                    rl = work.tile([128, F], f32, tag="rl")
                    nc.vector.tensor_scalar(
                        rl, accs[ic], r, 0.0, op0=OP.mult, op1=OP.max
                    )
                    ot = work.tile([128, F], f32, tag="ot")
                    nc.vector.scalar_tensor_tensor(
                        ot, ex, -1.0, rl, op0=OP.add, op1=OP.min
                    )
                    nc.sync.dma_start(out=out[ic * 128:(ic + 1) * 128, :], in_=ot)
    nc.compile()
    return nc


def _get(name):
    if name not in _CACHE:
        _CACHE[name] = {"k0": _build_k0, "k1": _build_k1, "k2": _build_k2}[name]()
    return _CACHE[name]


# ---------------------------------------------------------------- host
def kernel(x, left, adj, Ws, a1, a2, Wo, ao1, ao2):
    x = np.asarray(x, np.float32)
    adj = np.asarray(adj, np.float32)
    Ws = np.asarray(Ws, np.float32)
    a1 = np.asarray(a1, np.float32)
    a2 = np.asarray(a2, np.float32)
    Wo = np.asarray(Wo, np.float32)
    ao1 = np.asarray(ao1, np.float32)
    ao2 = np.asarray(ao2, np.float32)

    # host-side layout prep (no significant FLOPs)
    ws_all = np.ascontiguousarray(Ws.transpose(1, 0, 2).reshape(F, F))
    ws1 = np.einsum("hkf,hf->kh", Ws, a1)   # [F, H]  tiny matvecs
    ws2 = np.einsum("hkf,hf->kh", Ws, a2)
    wsa16 = ws_all.astype(np.float16)
    wsv16 = np.ascontiguousarray(
        np.concatenate([ws1, ws2], axis=1)
    ).astype(np.float16)
    woa = np.ascontiguousarray(Wo).astype(np.float16)
    wosv = np.ascontiguousarray(
        np.stack([Wo @ ao1, Wo @ ao2], axis=1), dtype=np.float32
    )
    adj16 = adj.astype(np.float16)  # exact: adj is a 0/1 mask
    adjt_c = [
        np.ascontiguousarray(adj16[c * R:(c + 1) * R].T) for c in range(M)
    ]
    xt_c = [np.ascontiguousarray(x[c * R:(c + 1) * R].T) for c in range(M)]

    cores = list(range(M))

    k0 = _get("k0")
    res0 = _run(
        k0,
        [
            {"xin": np.concatenate(
                [xt_c[c].astype(np.float16), wsa16, wsv16], axis=1
            )}
            for c in cores
        ],
        cores,
    )
    wha = np.concatenate([res0.results[c]["wha16"] for c in cores], axis=0)
    evf = np.concatenate([res0.results[c]["ev"] for c in cores], axis=0)

    def evt16_for(c):
        ev = evf[c * R:(c + 1) * R]
        out = np.zeros((32, R), np.float16)
        out[0:H] = ev[:, 2 * H:3 * H].T
        out[H:2 * H] = ev[:, 3 * H:4 * H].T - BIG
        return out

    bcd = np.zeros((32, 2 * H * 128), np.float16)
    bcd[0:2 * H] = np.repeat(np.eye(2 * H, dtype=np.float16), 128, axis=1)
    k1 = _get("k1")
    res1 = _run(
        k1,
        [
            {
                "wha": wha,
                "evf": evf,
                "evt16": evt16_for(c),
                "bcd": bcd,
                "adjt": adjt_c[c],
                "woa": woa,
                "wosv": wosv,
            }
            for c in cores
        ],
        cores,
    )
    whoa = np.concatenate([res1.results[c]["whoa16"] for c in cores], axis=0)
    svof = np.concatenate([res1.results[c]["svo"] for c in cores], axis=0)

    # k2 scalar prep (tiny): shifted exponentials of the output-layer scores
    s1o, s2o = svof[:, 0].astype(np.float64), svof[:, 1].astype(np.float64)
    bias = 9.0 - s2o.max()
    ef = np.empty((128, JB, 2), np.float32)
    ef[:, :, 0] = np.exp(s2o + bias).reshape(JB, 128).T
    ef[:, :, 1] = np.exp(0.2 * s2o + bias).reshape(JB, 128).T
    f1o = np.exp(-0.8 * s1o).astype(np.float16)

    k2 = _get("k2")
    res2 = _run(
        k2,
        [
            {
                "whoa": whoa,
                "ef": ef,
                "f1r": f1o[c * R:(c + 1) * R].reshape(1, R),
                "adjt": adjt_c[c],
            }
            for c in cores
        ],
        cores,
    )
    return np.concatenate([res2.results[c]["out"] for c in cores], axis=0)

